# revision 3
# baseline (speedup 1.0000x reference)
"""Block-causal multi-head attention (B=1, S=4096, E=1024, H=16, BLK=128) on 8 trn2 cores.

Strategy: head-parallel attention (2 heads/core) + sequence-parallel out_proj
(512 q-columns/core via fp16 AllToAll), everything in fp16:

  - On real HW a DoubleRow fp8 matmul streams N output columns in ~N cycles —
    the same as one fp16 matmul — so the old fp8 value+residual 3-term scheme
    (12 DR matmuls per [128,512] psum) was 1.5x SLOWER than 8 plain fp16
    matmuls. fp16 operands everywhere (QKV proj, scores, PV, out_proj) cut
    PE work ~25% and improve rel err to ~6e-4.
  - Scores^T [128k x 512q] per (key-block, q-group): two K=64 matmuls
    (head0 on partitions 0:64, head1 on 64:128) land on disjoint PE
    row-groups via auto tile_position and can run concurrently.
  - Head1 scores are packed adjacent to head0 ([GQ : 2GQ-qoff]) so each
    block's exp is ONE ACT instruction (measured ~143ns serial overhead
    per ACT instruction) with no stale-psum gap.
  - exp(0.125 s) on ACT writes fp16 P; PV in fp16 with V augmented with 64
    ones columns so acc psum partitions 64..127 hold the softmax
    denominator; normalize is pure DVE.
  - V is produced directly in [key-pos, dims] layout (x^T chunk stationary,
    Wv^T chunk moving, 4 seq-blocks per psum bank) — no PE transposes.
  - The rep loop is software-pipelined: each rep's out_proj matmuls are
    deferred into the next rep's attention stream as interleave items, so
    the PE never idles (or goes HAM-cold) across the ~13us AllToAll.
"""
import numpy as np

import concourse.bass as bass
import concourse.mybir as mybir
from concourse import bacc, tile
from concourse.bass_utils import run_bass_kernel_spmd

N_CORES = 8
S, E, H, BLK, D = 4096, 1024, 16, 128, 64
NB = S // BLK            # 32 key/query blocks
NG = 8                   # q-groups of 512
GQ = 512                 # q columns per group
HPC = H // N_CORES       # heads per core (2)
RPC = 3 * HPC * D        # in_proj rows per core (384)

F32 = mybir.dt.float32
F32R = mybir.dt.float32r
F16 = mybir.dt.float16
ALU = mybir.AluOpType
ACTF = mybir.ActivationFunctionType


def build_nc(reps: int = 1, cc: bool = True):
    nc = bacc.Bacc("TRN2", target_bir_lowering=False, debug=False, num_devices=N_CORES)

    xT = nc.dram_tensor("xT", [8, 128, S], F16, kind="ExternalInput")
    wq = nc.dram_tensor("wq", [8, 128, RPC], F16, kind="ExternalInput")
    bqkv = nc.dram_tensor("bqkv", [3, 2 * D], F32, kind="ExternalInput")
    vb = nc.dram_tensor("vb", [128, 2 * D], F16, kind="ExternalInput")
    wo = nc.dram_tensor("wo", [8, 128, E], F16, kind="ExternalInput")
    bout = nc.dram_tensor("bout", [8, 128], F32, kind="ExternalInput")
    yT = nc.dram_tensor("yT", [E, GQ], F32, kind="ExternalOutput")

    with tile.TileContext(nc) as tc:
        with (
            tc.tile_pool(name="const", bufs=1) as constp,
            tc.tile_pool(name="qkv", bufs=1) as qkvp,
            tc.tile_pool(name="xt", bufs=16) as xtp,
            tc.tile_pool(name="pt", bufs=6) as ptp,
            tc.tile_pool(name="small", bufs=4) as smallp,
            tc.tile_pool(name="attn", bufs=1) as attnp,
            tc.tile_pool(name="ytp", bufs=2) as ytp,
            tc.tile_pool(name="pp", bufs=2, space="PSUM") as pp,
            tc.tile_pool(name="scores", bufs=2, space="PSUM") as scp,
            tc.tile_pool(name="accum", bufs=2, space="PSUM") as accp,
            tc.tile_pool(name="dram", bufs=1, space="DRAM") as dram,
        ):
            # ---- constants / weights ----
            bvb = constp.tile([128, 2 * D], F16)
            bq_sb = constp.tile([128, 3], F32)
            bo_sb = constp.tile([128, 8], F32)
            wq_sb = constp.tile([128, 8 * RPC], F16)
            wq_v = wq_sb[:].rearrange("p (c r) -> p c r", c=8)

            def load_wq(c):
                nc.sync.dma_start(wq_v[:, c, :], wq.ap()[c])

            def load_biases():
                nc.sync.dma_start(bq_sb[:], bqkv.ap().rearrange("r p -> p r"))
                nc.sync.dma_start(bo_sb[:], bout.ap().rearrange("t p -> p t"))
                nc.sync.dma_start(bvb[:], vb.ap())

            wo_sb = constp.tile([128, 8 * E], F16)
            wo_v = wo_sb[:].rearrange("p (m e) -> p m e", m=8)

            def load_wout():
                for c in range(8):
                    nc.sync.dma_start(wo_v[:, c, :], wo.ap()[c])

            # persistent per-rep tensors; partitions 0:64 head0, 64:128 head1.
            qt = qkvp.tile([128, S], F16, tag="qt")
            kt = qkvp.tile([128, S], F16, tag="kt")
            v_sb = qkvp.tile([128, 2 * NB * 2 * D], F16, tag="vsb")
            v_view = v_sb[:].rearrange("p (h b dd) -> p h b dd", h=2, b=NB)
            cc_in = dram.tile([N_CORES, 128, GQ], F16, tag="ccin")
            cc_out = dram.tile([N_CORES, 128, GQ], F16, tag="ccout")

            backlog = []
            for rep in range(reps):
                # ones columns of V (denominator trick); rewritten each rep
                if rep == 0:
                    nc.vector.memset(v_view[:, :, :, D:2 * D], 1.0)

                # ---------- proj work-item machinery ----------
                def xt_dmas(g):
                    xts = []
                    for c in range(8):
                        if g == 0 and rep == 0:
                            load_wq(c)      # interleave weight chunks with first x tiles
                        xt = xtp.tile([128, GQ], F16, tag="xt")
                        nc.sync.dma_start(
                            xt[:], xT.ap()[c][:, g * GQ:(g + 1) * GQ])
                        xts.append(xt)
                    if g == 0 and rep == 0:
                        load_biases()
                    return xts

                def proj_items(g, xts):
                    """Yield closures emitting proj instructions for group g."""
                    sl = slice(g * GQ, (g + 1) * GQ)

                    def rtile(which):
                        ps = pp.tile([128, GQ], F32, tag="pp")
                        msl = slice(which * 128, (which + 1) * 128)
                        for c in range(8):
                            yield lambda c=c, ps=ps: nc.tensor.matmul(
                                ps[:], wq_v[:, c, msl], xts[c][:],
                                start=(c == 0), stop=(c == 7))
                        if which == 0:      # q
                            yield lambda ps=ps: nc.vector.tensor_scalar(
                                qt[:, sl], ps[:], bq_sb[:, 0:1], None, ALU.add)
                        else:               # k
                            yield lambda ps=ps: nc.vector.tensor_scalar(
                                kt[:, sl], ps[:], bq_sb[:, 1:2], None, ALU.add)

                    def vtile():
                        # v computed directly in [key-pos, dims] layout:
                        # lhsT = x^T chunk (stationary), rhs = Wv^T chunk.
                        # Four seq-blocks share one psum bank as quarters.
                        ps = pp.tile([128, GQ], F32, tag="pp")
                        for j in range(4):
                            jsl = slice(j * 128, (j + 1) * 128)
                            for c in range(8):
                                yield lambda c=c, j=j, ps=ps, jsl=jsl: \
                                    nc.tensor.matmul(
                                        ps[:, jsl], xts[c][:, jsl],
                                        wq_v[:, c, 256:384],
                                        start=(c == 0), stop=(c == 7),
                                        skip_group_check=True)

                            def vcopy(j=j, ps=ps, jsl=jsl, bk=4 * g + j):
                                nc.vector.tensor_tensor(
                                    v_view[:, :, bk, 0:D],
                                    ps[:, jsl].rearrange("p (h d) -> p h d", h=2),
                                    bvb[:].rearrange("p (h d) -> p h d", h=2),
                                    ALU.add)
                            yield vcopy
                    yield from rtile(0)
                    yield from rtile(1)
                    yield from vtile()

                def attention_group(g, pending, backlog):
                    """Emit attention for q-group g, interleaving `pending` proj
                    items (must drain by group end) and `backlog` items (the
                    previous rep's out_proj; no deadline, popped spare-time)."""
                    nbk = 4 * g + 4
                    CAP = 3
                    # throttle interleaved proj items in the first two blocks so the
                    # group's exp pipeline primes before PE picks up filler work
                    quota = []
                    rem = len(pending)
                    for i in range(nbk):
                        if i < 2:
                            q = min(rem, 1)
                        else:
                            left = nbk - i
                            q = (rem + left - 1) // left
                        quota.append(q)
                        rem -= q
                    pt_tiles = {}
                    acc_a = accp.tile([128, GQ], F32, tag="acc")
                    acc_b = accp.tile([128, GQ], F32, tag="acc")
                    for bk in range(nbk):
                        qoff = max(0, (bk - 4 * g)) * 128
                        sc = scp.tile([128, 2 * GQ], F32, tag="sc")
                        ksl = slice(bk * 128, (bk + 1) * 128)
                        qsl = slice(g * GQ + qoff, (g + 1) * GQ)
                        # head0 -> [qoff:GQ]; head1 packed adjacent at
                        # [GQ : 2GQ-qoff] so exp is one gapless instruction.
                        nc.tensor.matmul(
                            sc[:, qoff:GQ],
                            kt[0:64, ksl], qt[0:64, qsl],
                            start=True, stop=True, skip_group_check=True)
                        nc.tensor.matmul(
                            sc[:, GQ:2 * GQ - qoff],
                            kt[64:128, ksl], qt[64:128, qsl],
                            start=True, stop=True, skip_group_check=True)
                        pt = ptp.tile([128, 2 * GQ], F16, tag="pt")
                        nc.scalar.activation(pt[:, qoff:2 * GQ - qoff],
                                             sc[:, qoff:2 * GQ - qoff],
                                             ACTF.Exp, scale=0.125)
                        pt_tiles[bk] = (pt, qoff)
                        # PV for the previous block (keeps PE busy while ACT exps)
                        if bk > 0:
                            emit_pv(g, bk - 1, pt_tiles, acc_a, acc_b)
                        popped = 0
                        for _ in range(quota[bk]):
                            if pending:
                                pending.pop(0)()
                                popped += 1
                        if g >= 1 and bk >= 2:
                            # (group 0 is too early: the previous rep's
                            # all-to-all is still in flight)
                            while backlog and popped < CAP:
                                backlog.pop(0)()
                                popped += 1
                    emit_pv(g, nbk - 1, pt_tiles, acc_a, acc_b, last=True)
                    while pending:
                        pending.pop(0)()
                    return normalize_items(g, acc_a, acc_b)

                def normalize_items(g, acc_a, acc_b):
                    # deferred normalize + all-to-all staging closures for group g.
                    # acc rows 64:128 all hold the softmax denominator (64 ones
                    # columns in V), so normalize is partition-aligned DVE work.
                    items = []
                    for h, acc in ((0, acc_a), (1, acc_b)):
                        def norm(h=h, acc=acc, g=g):
                            recip = smallp.tile([64, GQ], F32R, tag="recip")
                            with nc.allow_low_precision(reason="softmax denom reciprocal in fp32r"):
                                nc.vector.reciprocal(recip[:], acc[64:128, :])
                            at16 = smallp.tile([64, GQ], F16, tag="at16")
                            nc.vector.tensor_tensor(at16[:], acc[0:64, :], recip[:], ALU.mult)
                            nc.gpsimd.dma_start(cc_in[:][g, h * 64:(h + 1) * 64, :], at16[:])
                        items.append(norm)
                    return items

                def emit_pv(g, bk, pt_tiles, acc_a, acc_b, last=False):
                    pt, qoff = pt_tiles.pop(bk)
                    nc.tensor.matmul(
                        acc_a[:, qoff:GQ], v_view[:, 0, bk, :], pt[:, qoff:GQ],
                        start=(bk == 0), stop=last, skip_group_check=True)
                    nc.tensor.matmul(
                        acc_b[:, qoff:GQ], v_view[:, 1, bk, :],
                        pt[:, GQ:2 * GQ - qoff],
                        start=(bk == 0), stop=last, skip_group_check=True)

                def out_proj_items():
                    # all-to-all + unpack now; out_proj matmuls as closures so
                    # the NEXT rep's attention stream absorbs them (PE stays
                    # busy and warm through the collective latency).
                    if cc:
                        nc.gpsimd.collective_compute(
                            "AllToAll", ALU.bypass,
                            replica_groups=[list(range(N_CORES))],
                            ins=[cc_in.opt()], outs=[cc_out.opt()])
                    else:
                        nc.gpsimd.dma_start(cc_out[:], cc_in[:])
                    at_sb = attnp.tile([128, 8 * GQ], F16, tag="at")
                    at_v = at_sb[:].rearrange("p (m q) -> p m q", m=8)
                    for j in range(N_CORES):
                        nc.gpsimd.dma_start(at_v[:, j, :], cc_out[:][j])
                    items = []
                    for t in range(8):
                        # psum tile allocated lazily at first matmul so pool
                        # buffer-reuse deps line up with actual pop order.
                        box = {}
                        tsl = slice(t * 128, (t + 1) * 128)

                        def mm(m, t=t, box=box, tsl=tsl):
                            if m == 0:
                                box["ps"] = pp.tile([128, GQ], F32, tag="pp",
                                                    name="ops")
                            nc.tensor.matmul(
                                box["ps"][:], wo_v[:, m, tsl], at_v[:, m, :],
                                start=(m == 0), stop=(m == 7))
                        for m in range(8):
                            items.append(lambda m=m, mm=mm: mm(m))

                        def tail(t=t, box=box, tsl=tsl):
                            yt = ytp.tile([128, GQ], F32, tag="yt")
                            nc.vector.tensor_scalar(yt[:], box["ps"][:],
                                                    bo_sb[:, t:t + 1],
                                                    None, ALU.add)
                            nc.sync.dma_start(yT.ap()[tsl, :], yt[:])
                        items.append(tail)
                    return items

                # ---------- emit: proj(0), attention groups, pipelined epilogue ----------
                xts0 = xt_dmas(0)
                for item in proj_items(0, xts0):
                    item()
                carry = []
                for g in range(NG):
                    if g + 1 < NG:
                        nxt = xt_dmas(g + 1)
                        pending = carry + list(proj_items(g + 1, nxt))
                    else:
                        pending = carry
                    carry = attention_group(g, pending, backlog)
                    if g == 1 and rep == 0:
                        load_wout()
                for item in carry:
                    item()
                while backlog:      # leftover of previous rep's out_proj
                    backlog.pop(0)()
                backlog = out_proj_items()
                if rep == reps - 1:
                    while backlog:
                        backlog.pop(0)()

    nc.compile()
    return nc


_NC_CACHE = {}


def _get_nc(reps=1):
    if reps not in _NC_CACHE:
        _NC_CACHE[reps] = build_nc(reps)
    return _NC_CACHE[reps]


def make_in_maps(x, in_proj_weight, in_proj_bias, out_proj_weight, out_proj_bias):
    x = np.asarray(x, np.float32)
    w_in = np.asarray(in_proj_weight, np.float32)
    b_in = np.asarray(in_proj_bias, np.float32)
    w_out = np.asarray(out_proj_weight, np.float32)
    b_out = np.asarray(out_proj_bias, np.float32)

    xT = np.ascontiguousarray(x.reshape(S, E).T.reshape(8, 128, S)).astype(np.float16)
    woT = np.ascontiguousarray(w_out.T.reshape(8, 128, E)).astype(np.float16)
    bout = np.ascontiguousarray(b_out.reshape(8, 128))
    in_maps = []
    for c in range(N_CORES):
        rows = []
        for blk in range(3):  # q, k, v blocks of in_proj
            for h in (2 * c, 2 * c + 1):
                rows.extend(range(blk * E + h * D, blk * E + (h + 1) * D))
        rows = np.array(rows)
        wqT = np.ascontiguousarray(
            w_in[rows].T.reshape(8, 128, RPC)).astype(np.float16)  # [1024, 384]
        bqkv = np.ascontiguousarray(b_in[rows].reshape(3, 2 * D))
        vbb = np.ascontiguousarray(
            np.broadcast_to(bqkv[2], (128, 2 * D))).astype(np.float16)
        in_maps.append({
            "xT": xT, "wq": wqT, "bqkv": bqkv, "wo": woT, "bout": bout,
            "vb": vbb,
        })
    return in_maps


def assemble_output(results):
    yT_full = np.concatenate([results[c]["yT"] for c in range(N_CORES)], axis=1)
    return np.ascontiguousarray(yT_full.T).reshape(1, S, E).astype(np.float32)


def kernel(x, in_proj_weight, in_proj_bias, out_proj_weight, out_proj_bias,
           block_size, num_heads):
    assert int(np.asarray(block_size)) == BLK and int(np.asarray(num_heads)) == H
    in_maps = make_in_maps(x, in_proj_weight, in_proj_bias,
                           out_proj_weight, out_proj_bias)
    nc = _get_nc(1)
    res = run_bass_kernel_spmd(nc, in_maps, core_ids=list(range(N_CORES)))
    return assemble_output(res.results)
